# revision 8
# baseline (speedup 1.0000x reference)
"""Trainium2 Bass kernel for the spiking-dense first-crossing problem.

Computes out[n,y] = min(1 + argmax_t(V[t,n,y] > 1), 64) where
V[t] = (spike mask up to t) @ weight, via one big masked matmul:

  V^T[(y), (n,t)] = W_slice^T @ mask   (W stationary, y on PSUM partitions)

All-bf16 datapath: spike times are pre-ceiled on host so they are exact
integers in bf16 (mask compare unchanged), weight is rounded to bf16 and
any element whose |V-1| margin is below FIX_EPS is recomputed exactly on
host from the full-precision weight (same margin-fixup scheme as the
f32r variant, slightly larger eps).

First-crossing extraction per PSUM bank: one DVE scalar_tensor_tensor
z = (V > 1) * (T - t), reduce_max -> rm; final out = 65 - max(rm, 1)
in one ACT pass over all y-tiles. Margin: ACT |V-1| -> DVE reduce_min.

Sharding: 2-way over Y (output cols) x 4-way over batch N across the 8
NeuronCores; each core computes a (1024 y, 16 n) block of out^T. The
full weight column-slice (2048 x 1024, bf16) stays resident in SBUF;
its load is spread over the sync/scalar/gpsimd DMA queues so the first
contraction chunk lands early. Mask chunks are built on DVE in f0/f1
halves so the first matmul only waits for 512 columns.
"""
import os
import sys
import numpy as np

for _p in ('/opt/trn_rl_repo',):
    if os.path.isdir(_p) and _p not in sys.path:
        sys.path.append(_p)

X, T, NN, YY = 2048, 64, 64, 2048
Y_SH, N_SH = 2, 4
YC = YY // Y_SH          # 1024 y-cols per core
NCB = NN // N_SH         # 16 batch rows per core
KC = X // 128            # 16 contraction chunks
FT = NCB * T             # 1024 mask free cols per core
NFT = FT // 512          # 2 f-tiles (512 = 8 n x 64 t)
NPF = 512 // T           # 8 n's per f-tile
NYT = YC // 128          # 8 y-tiles

AUXC = KC * NCB + 2 * T  # aux columns (bf16): [inT | tb | revt]

FIX_EPS = 8e-3  # host-recompute elements with |V-1| margin below this
TRACE = False

_cache = {}
LAST_RESULTS = None


def _ensure_ntff_hook():
    """Register the axon NTFF profiling hook if the environment lacks
    antenv.axon_hooks (the slim agent image) but has trn_agent_boot.
    Only adds capability; no-op when the real module exists."""
    try:
        import antenv.axon_hooks  # noqa: F401
        return
    except ImportError:
        pass
    try:
        import types
        from trn_agent_boot.trn_boot import _ntff_profile_via_ctypes
        hook = _ntff_profile_via_ctypes('/opt/axon/libaxon_pjrt.so')
        if hook is None:
            return
        import antenv
        mod = types.ModuleType('antenv.axon_hooks')
        mod.get_axon_ntff_profile_hook = lambda: hook
        mod.set_axon_ntff_profile_hook = lambda h: None
        sys.modules['antenv.axon_hooks'] = mod
        antenv.axon_hooks = mod
    except Exception:
        pass


def _safe_upload_artifacts():
    """upload_artifacts needs a bucket; make it degrade to a no-op path
    so tracing works in sandboxes without one."""
    try:
        from concourse import bass_utils
        orig = bass_utils.upload_artifacts
        if getattr(bass_utils, "_ul_wrapped", False):
            return
        def wrapped(tmpdir):
            try:
                return orig(tmpdir)
            except Exception:
                return str(tmpdir)
        bass_utils.upload_artifacts = wrapped
        bass_utils._ul_wrapped = True
    except Exception:
        pass


def _build_nc(reps=1):
    import concourse.bacc as bacc
    import concourse.mybir as mybir
    import concourse.tile as tile

    dt = mybir.dt
    f32 = dt.float32
    bf16 = dt.bfloat16
    nc = bacc.Bacc("TRN2", target_bir_lowering=False, debug=False)

    w_d = nc.dram_tensor("w", (X, YC), bf16, kind="ExternalInput")
    aux_d = nc.dram_tensor("aux", (128, AUXC), bf16, kind="ExternalInput")
    # obuf: [out (NYT*NCB) | margin (NYT*NCB)] so one DMA drains both
    obuf_d = nc.dram_tensor("obuf", (128, 2 * NYT * NCB), f32,
                            kind="ExternalOutput")

    with tile.TileContext(nc) as tc:
        with tc.tile_pool(name="const", bufs=1) as cpool, \
             tc.tile_pool(name="wp", bufs=1) as wpool, \
             tc.tile_pool(name="mp", bufs=1) as mpool, \
             tc.tile_pool(name="ps", bufs=8, space="PSUM") as ps, \
             tc.tile_pool(name="sz", bufs=6) as szpool, \
             tc.tile_pool(name="po", bufs=1) as popool:
            # PE warmup: short bf16 matmuls on junk data keep the PE busy
            # through the startup DMA window so HAM un-throttles before
            # the first real matmul arrives.
            junk_sb = cpool.tile([128, 512], bf16, tag="junk")
            nc.gpsimd.memset(junk_sb, 1.0)
            neg1_sb = cpool.tile([128, 1], f32, tag="neg1")
            nc.vector.memset(neg1_sb, -1.0)
            warm_pt = ps.tile([128, 128], f32, tag="pt", name="warm_pt")
            for _ in range(16):
                nc.tensor.matmul(warm_pt, junk_sb[:, 0:128],
                                 junk_sb[:, 0:128], start=True, stop=True)

            for rep in range(reps):
                aux_sb = cpool.tile([128, AUXC], bf16, tag="aux")
                nc.sync.dma_start(out=aux_sb, in_=aux_d.ap())
                inT_sb = aux_sb[:, 0:KC * NCB]
                tb_sb = aux_sb[:, KC * NCB:KC * NCB + T]
                revt_sb = aux_sb[:, KC * NCB + T:KC * NCB + 2 * T]

                # weight chunks, resident; loads spread over three DMA
                # queues, first chunks alone so they land early
                groups = [[0], [1], [2, 3], [4, 5, 6, 7],
                          [8, 9, 10, 11], [12, 13, 14, 15]]
                qeng = [nc.scalar, nc.gpsimd, nc.sync]
                w_tiles = [None] * KC
                for gi, grp in enumerate(groups):
                    gt = wpool.tile([128, len(grp) * YC], bf16, tag=f"wg{gi}")
                    qeng[gi % 3].dma_start(
                        out=gt[:].rearrange("p (g y) -> p g y", g=len(grp)),
                        in_=w_d.ap()[grp[0] * 128:
                                     (grp[-1] + 1) * 128, :].rearrange(
                                         "(g q) y -> q g y", g=len(grp)))
                    for j, k in enumerate(grp):
                        w_tiles[k] = gt[:, j * YC:(j + 1) * YC]

                mask_tiles = [mpool.tile([128, FT], bf16, tag=f"m{k}",
                                         name=f"mask{k}")
                              for k in range(KC)]
                rm_sh = popool.tile([128, NYT * NCB], bf16, tag="rmsh")
                obuf_sb = popool.tile([128, 2 * NYT * NCB], f32, tag="obuf")
                mg_view = obuf_sb[:, NYT * NCB:2 * NYT * NCB]

                def emit_mask(k, f):
                    nsl = slice(f * NPF, (f + 1) * NPF)
                    t_b = tb_sb.unsqueeze(1).broadcast_to((128, NPF, T))
                    s_b = inT_sb[:, k * NCB:(k + 1) * NCB][:, nsl] \
                        .unsqueeze(2).broadcast_to((128, NPF, T))
                    nc.vector.tensor_tensor(
                        mask_tiles[k][:, f * 512:(f + 1) * 512].rearrange(
                            "p (n t) -> p n t", n=NPF),
                        t_b, s_b, mybir.AluOpType.is_ge)

                def emit_mm(pt, k, yt, f):
                    rhs = mask_tiles[k][:, f * 512:(f + 1) * 512]
                    lhsT = w_tiles[k][:, yt * 128:(yt + 1) * 128]
                    nc.tensor.matmul(pt, lhsT, rhs,
                                     start=(k == 0), stop=(k == KC - 1))

                def emit_post(pt, yt, f):
                    # rm = max_t (V > 1) * (T - t); 0 when never crossed
                    z_t = szpool.tile([128, 512], bf16, tag="z")
                    r_b = revt_sb.unsqueeze(1).broadcast_to((128, NPF, T))
                    nc.vector.scalar_tensor_tensor(
                        z_t[:].rearrange("p (n t) -> p n t", n=NPF),
                        pt[:].rearrange("p (n t) -> p n t", n=NPF),
                        1.0, r_b,
                        mybir.AluOpType.is_gt, mybir.AluOpType.mult)
                    nc.vector.tensor_reduce(
                        rm_sh[:, yt * NCB + f * NPF:yt * NCB + (f + 1) * NPF],
                        z_t[:].rearrange("p (n t) -> p n t", n=NPF),
                        axis=mybir.AxisListType.X, op=mybir.AluOpType.max)
                    # margin = min_t |V - 1| on ACT + DVE
                    a_t = szpool.tile([128, 512], f32, tag="a")
                    nc.scalar.activation(a_t, pt,
                                         mybir.ActivationFunctionType.Abs,
                                         bias=neg1_sb[:])
                    nc.vector.tensor_reduce(
                        mg_view[:, yt * NCB + f * NPF:yt * NCB + (f + 1) * NPF],
                        a_t[:].rearrange("p (n t) -> p n t", n=NPF),
                        axis=mybir.AxisListType.X, op=mybir.AluOpType.min)

                # f0-half masks first (chunk order = consumption order),
                # f1 halves after; DVE stays ahead of the PE throughout.
                for k in range(KC):
                    emit_mask(k, 0)
                for k in range(KC):
                    emit_mask(k, 1)

                # f0 pass: k-outer so the PE trails the mask builder,
                # y-tiles 0..6 accumulate in 7 PSUM banks.
                pts = []
                for k in range(KC):
                    for yt in range(NYT - 1):
                        if k == 0:
                            pts.append(ps.tile([128, 512], f32, tag="pt",
                                               name=f"pt0_{yt}"))
                        emit_mm(pts[yt], k, yt, 0)
                # yt7-f0 rolls straight on (bank = recycled warmup slot)
                # while yts 0..6 postproc drains their banks; yt7-f1 then
                # lands in yt0's freed bank with no PE stall.
                pt7 = [ps.tile([128, 512], f32, tag="pt", name="pt7_0")]
                for k in range(KC):
                    emit_mm(pt7[0], k, NYT - 1, 0)
                for yt in range(NYT - 1):
                    emit_post(pts[yt], yt, 0)
                pt7.append(ps.tile([128, 512], f32, tag="pt", name="pt7_1"))
                for k in range(KC):
                    emit_mm(pt7[1], k, NYT - 1, 1)
                for f in range(2):
                    emit_post(pt7[f], NYT - 1, f)

                # f1 pass: y-outer so banks finish staggered and postproc
                # overlaps later y-tiles.
                for yt in range(NYT - 1):
                    pt = ps.tile([128, 512], f32, tag="pt", name=f"pt1_{yt}")
                    for k in range(KC):
                        emit_mm(pt, k, yt, 1)
                    emit_post(pt, yt, 1)

                # out = 65 - max(rm, 1): crossed at t -> t+1, never -> 64
                nc.vector.tensor_scalar_max(rm_sh[:], rm_sh[:], 1.0)
                nc.scalar.activation(
                    obuf_sb[:, 0:NYT * NCB], rm_sh[:],
                    mybir.ActivationFunctionType.Copy,
                    bias=float(T + 1), scale=-1.0)
                nc.sync.dma_start(out=obuf_d.ap(), in_=obuf_sb)

    nc.compile()
    return nc


def _make_in_maps(inputs):
    import ml_dtypes

    input = np.ascontiguousarray(np.asarray(inputs["input"], dtype=np.float32))
    weight = np.ascontiguousarray(np.asarray(inputs["weight"], dtype=np.float32))
    t_series = np.asarray(inputs["t_series"], dtype=np.float32).reshape(-1)

    s_ceil = np.ceil(input).astype(np.float32)   # exact in bf16 (ints <= 64)
    TB = np.tile(t_series, (128, 1)).astype(np.float32)
    REVT = np.tile((np.float32(T) - np.arange(T, dtype=np.float32)), (128, 1))

    in_maps = []
    for c in range(8):
        yb, nb = c % Y_SH, c // Y_SH
        wsl = np.ascontiguousarray(
            weight[:, yb * YC:(yb + 1) * YC]).astype(ml_dtypes.bfloat16)
        scl = s_ceil[nb * NCB:(nb + 1) * NCB, :]          # (NCB, X)
        inT = scl.reshape(NCB, KC, 128).transpose(2, 1, 0).reshape(128, KC * NCB)
        aux = np.ascontiguousarray(
            np.concatenate([inT, TB, REVT], axis=1)).astype(ml_dtypes.bfloat16)
        in_maps.append({"aux": aux, "w": wsl})
    return in_maps


def kernel(input, weight, t_series, T=64, **unused):
    global LAST_RESULTS
    from concourse import bass_utils

    _ensure_ntff_hook()
    _safe_upload_artifacts()
    if "nc" not in _cache:
        _cache["nc"] = _build_nc()
    nc = _cache["nc"]

    _cache["t_series"] = np.asarray(t_series, dtype=np.float32).reshape(-1)
    in_maps = _make_in_maps(
        {"input": input, "weight": weight, "t_series": t_series})

    res = bass_utils.run_bass_kernel_spmd(
        nc, in_maps, core_ids=list(range(8)), trace=TRACE)
    LAST_RESULTS = res

    # device layout: obuf[p, yt*NCB + n] = out for y = yt*128+p, batch n;
    # columns NYT*NCB.. hold the margins in the same layout
    O = np.empty((YY, NN), dtype=np.float32)
    M = np.empty((YY, NN), dtype=np.float32)
    for c, r in enumerate(res.results):
        yb, nb = c % Y_SH, c // Y_SH
        ob = r["obuf"].reshape(128, 2, NYT, NCB)
        O[yb * YC:(yb + 1) * YC, nb * NCB:(nb + 1) * NCB] = \
            ob[:, 0].transpose(1, 0, 2).reshape(YC, NCB)
        M[yb * YC:(yb + 1) * YC, nb * NCB:(nb + 1) * NCB] = \
            ob[:, 1].transpose(1, 0, 2).reshape(YC, NCB)
    out = np.ascontiguousarray(O.T)

    _host_fixup(out, M.T, np.asarray(input, np.float32),
                np.asarray(weight, np.float32))
    return out


def _host_fixup(out, margin, input, weight):
    """Recompute exactly (fp64) every element whose bf16 |V-1| margin is
    within the bf16 matmul error bound; in-place on `out`."""
    flags = margin < FIX_EPS
    if not flags.any():
        return
    # first step index j with t_series[j] >= in; == T means never spikes
    s = np.searchsorted(_cache.get("t_series", np.arange(T, dtype=np.float32)),
                        input, side="left").astype(np.int64)
    s = np.clip(s, 0, T)
    w64 = weight.astype(np.float64)
    for n in np.unique(np.nonzero(flags)[0]):
        ys = np.nonzero(flags[n])[0]
        d = np.zeros((T + 1, len(ys)))
        np.add.at(d, s[n], w64[:, ys])           # scatter rows by spike step
        V = np.cumsum(d[:T], axis=0)
        c = V > 1.0
        any_c = c.any(axis=0)
        idx = np.argmax(c, axis=0)
        out[n, ys] = np.where(any_c, idx + 1, T).astype(np.float32)


# revision 12
# speedup vs baseline: 1.0742x; 1.0742x over previous
"""Trainium2 Bass kernel for the spiking-dense first-crossing problem.

Computes out[n,y] = min(1 + argmax_t(V[t,n,y] > 1), 64) where
V[t] = (spike mask up to t) @ weight, via one big masked matmul:

  V^T[(y), (n,t)] = W_slice^T @ mask   (W stationary, y on PSUM partitions)

All-bf16 datapath: spike times are pre-ceiled on host so they are exact
integers in bf16 (mask compare unchanged), weight is rounded to bf16 and
any element whose |V-1| margin is below FIX_EPS is recomputed exactly on
host from the full-precision weight (same margin-fixup scheme as the
f32r variant, slightly larger eps).

First-crossing extraction per PSUM bank: one DVE scalar_tensor_tensor
z = (V > 1) * (T - t), reduce_max -> rm; final out = 65 - max(rm, 1)
in one ACT pass over all y-tiles. Margin: ACT |V-1| -> DVE reduce_min.

Sharding: 2-way over Y (output cols) x 4-way over batch N across the 8
NeuronCores; each core computes a (1024 y, 16 n) block of out^T. The
full weight column-slice (2048 x 1024, bf16) stays resident in SBUF;
its load is spread over the sync/scalar/gpsimd DMA queues so the first
contraction chunk lands early. Mask chunks are built on DVE in f0/f1
halves so the first matmul only waits for 512 columns.
"""
import os
import sys
import numpy as np

for _p in ('/opt/trn_rl_repo',):
    if os.path.isdir(_p) and _p not in sys.path:
        sys.path.append(_p)

X, T, NN, YY = 2048, 64, 64, 2048
Y_SH, N_SH = 2, 4
YC = YY // Y_SH          # 1024 y-cols per core
NCB = NN // N_SH         # 16 batch rows per core
KC = X // 128            # 16 contraction chunks
FT = NCB * T             # 1024 mask free cols per core
NFT = FT // 512          # 2 f-tiles (512 = 8 n x 64 t)
NPF = 512 // T           # 8 n's per f-tile
NYT = YC // 128          # 8 y-tiles

AUXC = KC * NCB + 2 * T  # aux columns (bf16): [inT | tb | revt]

FIX_EPS = 8e-3  # host-recompute elements with |V-1| margin below this
TRACE = False

_cache = {}
LAST_RESULTS = None


def _ensure_ntff_hook():
    """Register the axon NTFF profiling hook if the environment lacks
    antenv.axon_hooks (the slim agent image) but has trn_agent_boot.
    Only adds capability; no-op when the real module exists."""
    try:
        import antenv.axon_hooks  # noqa: F401
        return
    except ImportError:
        pass
    try:
        import types
        from trn_agent_boot.trn_boot import _ntff_profile_via_ctypes
        hook = _ntff_profile_via_ctypes('/opt/axon/libaxon_pjrt.so')
        if hook is None:
            return
        import antenv
        mod = types.ModuleType('antenv.axon_hooks')
        mod.get_axon_ntff_profile_hook = lambda: hook
        mod.set_axon_ntff_profile_hook = lambda h: None
        sys.modules['antenv.axon_hooks'] = mod
        antenv.axon_hooks = mod
    except Exception:
        pass


def _safe_upload_artifacts():
    """upload_artifacts needs a bucket; make it degrade to a no-op path
    so tracing works in sandboxes without one."""
    try:
        from concourse import bass_utils
        orig = bass_utils.upload_artifacts
        if getattr(bass_utils, "_ul_wrapped", False):
            return
        def wrapped(tmpdir):
            try:
                return orig(tmpdir)
            except Exception:
                return str(tmpdir)
        bass_utils.upload_artifacts = wrapped
        bass_utils._ul_wrapped = True
    except Exception:
        pass


def _build_nc(reps=1):
    import concourse.bacc as bacc
    import concourse.mybir as mybir
    import concourse.tile as tile

    dt = mybir.dt
    f32 = dt.float32
    bf16 = dt.bfloat16
    nc = bacc.Bacc("TRN2", target_bir_lowering=False, debug=False)

    w_d = nc.dram_tensor("w", (X, YC), bf16, kind="ExternalInput")
    aux_d = nc.dram_tensor("aux", (128, AUXC), bf16, kind="ExternalInput")
    # obuf: [out (NYT*NCB) | margin (NYT*NCB)] so one DMA drains both
    obuf_d = nc.dram_tensor("obuf", (128, 2 * NYT * NCB), f32,
                            kind="ExternalOutput")

    with tile.TileContext(nc) as tc:
        with tc.tile_pool(name="const", bufs=1) as cpool, \
             tc.tile_pool(name="wp", bufs=1) as wpool, \
             tc.tile_pool(name="mp", bufs=1) as mpool, \
             tc.tile_pool(name="ps", bufs=8, space="PSUM") as ps, \
             tc.tile_pool(name="sz", bufs=6) as szpool, \
             tc.tile_pool(name="po", bufs=1) as popool:
            # PE warmup: short bf16 matmuls on junk data keep the PE busy
            # through the startup DMA window so HAM un-throttles before
            # the first real matmul arrives.
            junk_sb = cpool.tile([128, 128], bf16, tag="junk")
            nc.vector.memset(junk_sb, 1.0)
            neg1_sb = cpool.tile([128, 1], f32, tag="neg1")
            nc.vector.memset(neg1_sb, -1.0)
            warm_pt = ps.tile([128, 128], f32, tag="pt", name="warm_pt")
            for _ in range(20):
                nc.tensor.matmul(warm_pt, junk_sb[:], junk_sb[:],
                                 start=True, stop=True)

            # warm each DMA queue with a tiny transfer: the first DMA on a
            # queue pays ~3us of cold-start latency before data lands, so
            # burn it on 2 bytes instead of the aux/weight payloads.
            scratch = cpool.tile([128, 4], bf16, tag="scr")
            for j, eng in enumerate((nc.gpsimd, nc.scalar, nc.sync)):
                eng.dma_start(out=scratch[0:1, j:j + 1],
                              in_=aux_d.ap()[0:1, 0:1])

            for rep in range(reps):
                aux_sb = cpool.tile([128, AUXC], bf16, tag="aux")
                nc.sync.dma_start(out=aux_sb, in_=aux_d.ap())
                inT_sb = aux_sb[:, 0:KC * NCB]
                tb_sb = aux_sb[:, KC * NCB:KC * NCB + T]
                revt_sb = aux_sb[:, KC * NCB + T:KC * NCB + 2 * T]

                # weight chunks, resident; loads round-robin over three DMA
                # queues in consumption order so no later chunk steals
                # bandwidth from an earlier one
                qeng = [nc.gpsimd, nc.scalar, nc.sync]
                w_tiles = []
                for k in range(KC):
                    tw = wpool.tile([128, YC], bf16, tag=f"w{k}")
                    qeng[k % 3].dma_start(
                        out=tw, in_=w_d.ap()[k * 128:(k + 1) * 128, :])
                    w_tiles.append(tw)

                mask_tiles = [mpool.tile([128, FT], bf16, tag=f"m{k}",
                                         name=f"mask{k}")
                              for k in range(KC)]
                rm_sh = popool.tile([128, NYT * NCB], bf16, tag="rmsh")
                obuf_sb = popool.tile([128, 2 * NYT * NCB], f32, tag="obuf")
                mg_view = obuf_sb[:, NYT * NCB:2 * NYT * NCB]

                def emit_mask(k, f):
                    nsl = slice(f * NPF, (f + 1) * NPF)
                    t_b = tb_sb.unsqueeze(1).broadcast_to((128, NPF, T))
                    s_b = inT_sb[:, k * NCB:(k + 1) * NCB][:, nsl] \
                        .unsqueeze(2).broadcast_to((128, NPF, T))
                    nc.vector.tensor_tensor(
                        mask_tiles[k][:, f * 512:(f + 1) * 512].rearrange(
                            "p (n t) -> p n t", n=NPF),
                        t_b, s_b, mybir.AluOpType.is_ge)

                def emit_mm(pt, k, yt, f):
                    rhs = mask_tiles[k][:, f * 512:(f + 1) * 512]
                    lhsT = w_tiles[k][:, yt * 128:(yt + 1) * 128]
                    nc.tensor.matmul(pt, lhsT, rhs,
                                     start=(k == 0), stop=(k == KC - 1))

                def emit_post(pt, yt, f):
                    # rm = max_t (V > 1) * (T - t); 0 when never crossed
                    z_t = szpool.tile([128, 512], bf16, tag="z")
                    r_b = revt_sb.unsqueeze(1).broadcast_to((128, NPF, T))
                    nc.vector.scalar_tensor_tensor(
                        z_t[:].rearrange("p (n t) -> p n t", n=NPF),
                        pt[:].rearrange("p (n t) -> p n t", n=NPF),
                        1.0, r_b,
                        mybir.AluOpType.is_gt, mybir.AluOpType.mult)
                    nc.vector.tensor_reduce(
                        rm_sh[:, yt * NCB + f * NPF:yt * NCB + (f + 1) * NPF],
                        z_t[:].rearrange("p (n t) -> p n t", n=NPF),
                        axis=mybir.AxisListType.X, op=mybir.AluOpType.max)
                    # margin = min_t |V - 1| on ACT + DVE
                    a_t = szpool.tile([128, 512], f32, tag="a")
                    nc.scalar.activation(a_t, pt,
                                         mybir.ActivationFunctionType.Abs,
                                         bias=neg1_sb[:])
                    nc.vector.tensor_reduce(
                        mg_view[:, yt * NCB + f * NPF:yt * NCB + (f + 1) * NPF],
                        a_t[:].rearrange("p (n t) -> p n t", n=NPF),
                        axis=mybir.AxisListType.X, op=mybir.AluOpType.min)

                # f0-half masks first (chunk order = consumption order),
                # f1 halves after; DVE stays ahead of the PE throughout.
                for k in range(KC):
                    emit_mask(k, 0)
                for k in range(KC):
                    emit_mask(k, 1)

                # f0 pass: k-outer so the PE trails the mask builder,
                # y-tiles 0..6 accumulate in 7 PSUM banks.
                pts = []
                for k in range(KC):
                    for yt in range(NYT - 1):
                        if k == 0:
                            pts.append(ps.tile([128, 512], f32, tag="pt",
                                               name=f"pt0_{yt}"))
                        emit_mm(pts[yt], k, yt, 0)
                # yt7-f0 rolls straight on (bank = recycled warmup slot)
                # while yts 0..6 postproc drains their banks; yt7-f1 then
                # lands in yt0's freed bank with no PE stall.
                pt7 = [ps.tile([128, 512], f32, tag="pt", name="pt7_0")]
                for k in range(KC):
                    emit_mm(pt7[0], k, NYT - 1, 0)
                for yt in range(NYT - 1):
                    emit_post(pts[yt], yt, 0)
                pt7.append(ps.tile([128, 512], f32, tag="pt", name="pt7_1"))
                for k in range(KC):
                    emit_mm(pt7[1], k, NYT - 1, 1)
                for f in range(2):
                    emit_post(pt7[f], NYT - 1, f)

                # f1 pass: y-outer so banks finish staggered and postproc
                # overlaps later y-tiles.
                for yt in range(NYT - 1):
                    pt = ps.tile([128, 512], f32, tag="pt", name=f"pt1_{yt}")
                    for k in range(KC):
                        emit_mm(pt, k, yt, 1)
                    emit_post(pt, yt, 1)

                # out = 65 - max(rm, 1): crossed at t -> t+1, never -> 64.
                # Drain out and margins as separate DMAs so the out half
                # doesn't wait on the final margin reduction.
                nc.vector.tensor_scalar_max(rm_sh[:], rm_sh[:], 1.0)
                nc.scalar.activation(
                    obuf_sb[:, 0:NYT * NCB], rm_sh[:],
                    mybir.ActivationFunctionType.Copy,
                    bias=float(T + 1), scale=-1.0)
                nc.sync.dma_start(out=obuf_d.ap()[:, 0:NYT * NCB],
                                  in_=obuf_sb[:, 0:NYT * NCB])
                nc.scalar.dma_start(out=obuf_d.ap()[:, NYT * NCB:],
                                    in_=mg_view)

    nc.compile()
    return nc


def _make_in_maps(inputs):
    import ml_dtypes

    input = np.ascontiguousarray(np.asarray(inputs["input"], dtype=np.float32))
    weight = np.ascontiguousarray(np.asarray(inputs["weight"], dtype=np.float32))
    t_series = np.asarray(inputs["t_series"], dtype=np.float32).reshape(-1)

    s_ceil = np.ceil(input).astype(np.float32)   # exact in bf16 (ints <= 64)
    TB = np.tile(t_series, (128, 1)).astype(np.float32)
    REVT = np.tile((np.float32(T) - np.arange(T, dtype=np.float32)), (128, 1))

    in_maps = []
    for c in range(8):
        yb, nb = c % Y_SH, c // Y_SH
        wsl = np.ascontiguousarray(
            weight[:, yb * YC:(yb + 1) * YC]).astype(ml_dtypes.bfloat16)
        scl = s_ceil[nb * NCB:(nb + 1) * NCB, :]          # (NCB, X)
        inT = scl.reshape(NCB, KC, 128).transpose(2, 1, 0).reshape(128, KC * NCB)
        aux = np.ascontiguousarray(
            np.concatenate([inT, TB, REVT], axis=1)).astype(ml_dtypes.bfloat16)
        in_maps.append({"aux": aux, "w": wsl})
    return in_maps


def kernel(input, weight, t_series, T=64, **unused):
    global LAST_RESULTS
    from concourse import bass_utils

    _ensure_ntff_hook()
    _safe_upload_artifacts()
    if "nc" not in _cache:
        _cache["nc"] = _build_nc()
    nc = _cache["nc"]

    _cache["t_series"] = np.asarray(t_series, dtype=np.float32).reshape(-1)
    in_maps = _make_in_maps(
        {"input": input, "weight": weight, "t_series": t_series})

    res = bass_utils.run_bass_kernel_spmd(
        nc, in_maps, core_ids=list(range(8)), trace=TRACE)
    LAST_RESULTS = res

    # device layout: obuf[p, yt*NCB + n] = out for y = yt*128+p, batch n;
    # columns NYT*NCB.. hold the margins in the same layout
    O = np.empty((YY, NN), dtype=np.float32)
    M = np.empty((YY, NN), dtype=np.float32)
    for c, r in enumerate(res.results):
        yb, nb = c % Y_SH, c // Y_SH
        ob = r["obuf"].reshape(128, 2, NYT, NCB)
        O[yb * YC:(yb + 1) * YC, nb * NCB:(nb + 1) * NCB] = \
            ob[:, 0].transpose(1, 0, 2).reshape(YC, NCB)
        M[yb * YC:(yb + 1) * YC, nb * NCB:(nb + 1) * NCB] = \
            ob[:, 1].transpose(1, 0, 2).reshape(YC, NCB)
    out = np.ascontiguousarray(O.T)

    _host_fixup(out, M.T, np.asarray(input, np.float32),
                np.asarray(weight, np.float32))
    return out


def _host_fixup(out, margin, input, weight):
    """Recompute exactly (fp64) every element whose bf16 |V-1| margin is
    within the bf16 matmul error bound; in-place on `out`."""
    flags = margin < FIX_EPS
    if not flags.any():
        return
    # first step index j with t_series[j] >= in; == T means never spikes
    s = np.searchsorted(_cache.get("t_series", np.arange(T, dtype=np.float32)),
                        input, side="left").astype(np.int64)
    s = np.clip(s, 0, T)
    w64 = weight.astype(np.float64)
    for n in np.unique(np.nonzero(flags)[0]):
        ys = np.nonzero(flags[n])[0]
        d = np.zeros((T + 1, len(ys)))
        np.add.at(d, s[n], w64[:, ys])           # scatter rows by spike step
        V = np.cumsum(d[:T], axis=0)
        c = V > 1.0
        any_c = c.any(axis=0)
        idx = np.argmax(c, axis=0)
        out[n, ys] = np.where(any_c, idx + 1, T).astype(np.float32)


# revision 23
# speedup vs baseline: 1.0782x; 1.0037x over previous
"""Trainium2 Bass kernel for the spiking-dense first-crossing problem.

Computes out[n,y] = min(1 + argmax_t(V[t,n,y] > 1), 64) where
V[t] = (spike mask up to t) @ weight, via one big masked matmul:

  V^T[(y), (n,t)] = W_slice^T @ mask   (W stationary, y on PSUM partitions)

All-bf16 datapath: spike times are pre-ceiled on host so they are exact
integers in bf16 (mask compare unchanged), weight is rounded to bf16 and
any element whose |V-1| margin is below FIX_EPS is recomputed exactly on
host from the full-precision weight (same margin-fixup scheme as the
f32r variant, slightly larger eps).

First-crossing extraction per PSUM bank: one DVE scalar_tensor_tensor
z = (V > 1) * (T - t), reduce_max -> rm; final out = 65 - max(rm, 1)
in one ACT pass over all y-tiles. Margin: ACT |V-1| -> DVE reduce_min.

Sharding: 2-way over Y (output cols) x 4-way over batch N across the 8
NeuronCores; each core computes a (1024 y, 16 n) block of out^T. The
full weight column-slice (2048 x 1024, bf16) stays resident in SBUF;
its load is spread over the sync/scalar/gpsimd DMA queues so the first
contraction chunk lands early. Mask chunks are built on DVE in f0/f1
halves so the first matmul only waits for 512 columns.
"""
import os
import sys
import numpy as np

for _p in ('/opt/trn_rl_repo',):
    if os.path.isdir(_p) and _p not in sys.path:
        sys.path.append(_p)

X, T, NN, YY = 2048, 64, 64, 2048
Y_SH, N_SH = 2, 4
YC = YY // Y_SH          # 1024 y-cols per core
NCB = NN // N_SH         # 16 batch rows per core
KC = X // 128            # 16 contraction chunks
FT = NCB * T             # 1024 mask free cols per core
NFT = FT // 512          # 2 f-tiles (512 = 8 n x 64 t)
NPF = 512 // T           # 8 n's per f-tile
NYT = YC // 128          # 8 y-tiles

AUXC = KC * NCB + 2 * T  # aux columns (bf16): [inT | tb | revt]

FIX_EPS = 1e-2  # host-recompute elements with |V-1| margin below this
                # (8e-3 true-margin bound + bf16 margin quantization slack)
TRACE = False

_cache = {}
LAST_RESULTS = None


def _ensure_ntff_hook():
    """Register the axon NTFF profiling hook if the environment lacks
    antenv.axon_hooks (the slim agent image) but has trn_agent_boot.
    Only adds capability; no-op when the real module exists."""
    try:
        import antenv.axon_hooks  # noqa: F401
        return
    except ImportError:
        pass
    try:
        import types
        from trn_agent_boot.trn_boot import _ntff_profile_via_ctypes
        hook = _ntff_profile_via_ctypes('/opt/axon/libaxon_pjrt.so')
        if hook is None:
            return
        import antenv
        mod = types.ModuleType('antenv.axon_hooks')
        mod.get_axon_ntff_profile_hook = lambda: hook
        mod.set_axon_ntff_profile_hook = lambda h: None
        sys.modules['antenv.axon_hooks'] = mod
        antenv.axon_hooks = mod
    except Exception:
        pass


def _safe_upload_artifacts():
    """upload_artifacts needs a bucket; make it degrade to a no-op path
    so tracing works in sandboxes without one."""
    try:
        from concourse import bass_utils
        orig = bass_utils.upload_artifacts
        if getattr(bass_utils, "_ul_wrapped", False):
            return
        def wrapped(tmpdir):
            try:
                return orig(tmpdir)
            except Exception:
                return str(tmpdir)
        bass_utils.upload_artifacts = wrapped
        bass_utils._ul_wrapped = True
    except Exception:
        pass


def _build_nc(reps=1):
    import concourse.bacc as bacc
    import concourse.mybir as mybir
    import concourse.tile as tile

    dt = mybir.dt
    f32 = dt.float32
    bf16 = dt.bfloat16
    nc = bacc.Bacc("TRN2", target_bir_lowering=False, debug=False)

    w_d = nc.dram_tensor("w", (X, YC), bf16, kind="ExternalInput")
    aux_d = nc.dram_tensor("aux", (128, AUXC), bf16, kind="ExternalInput")
    # out values are small integers and margins only gate the host fixup
    # threshold, so both are exact enough in bf16 (halves drain latency)
    obuf_d = nc.dram_tensor("obuf", (128, 2 * NYT * NCB), bf16,
                            kind="ExternalOutput")

    with tile.TileContext(nc) as tc:
        with tc.tile_pool(name="const", bufs=1) as cpool, \
             tc.tile_pool(name="wp", bufs=1) as wpool, \
             tc.tile_pool(name="mp", bufs=1) as mpool, \
             tc.tile_pool(name="ps", bufs=8, space="PSUM") as ps, \
             tc.tile_pool(name="sz", bufs=6) as szpool, \
             tc.tile_pool(name="po", bufs=1) as popool:
            # PE warmup: short bf16 matmuls on junk data keep the PE busy
            # through the startup DMA window so HAM un-throttles before
            # the first real matmul arrives.
            junk_sb = cpool.tile([128, 128], bf16, tag="junk")
            nc.vector.memset(junk_sb, 1.0)
            neg1_sb = cpool.tile([128, 1], f32, tag="neg1")
            nc.vector.memset(neg1_sb, -1.0)
            warm_pt = ps.tile([128, 128], f32, tag="pt", name="warm_pt")
            for _ in range(26):
                nc.tensor.matmul(warm_pt, junk_sb[:], junk_sb[:],
                                 start=True, stop=True)

            for rep in range(reps):
                aux_sb = cpool.tile([128, AUXC], bf16, tag="aux")
                nc.sync.dma_start(out=aux_sb, in_=aux_d.ap())
                inT_sb = aux_sb[:, 0:KC * NCB]
                tb_sb = aux_sb[:, KC * NCB:KC * NCB + T]
                revt_sb = aux_sb[:, KC * NCB + T:KC * NCB + 2 * T]

                # weight chunks, resident. Early DMAs run cold (~1.7us pipe
                # latency + ~95GB/s per queue), so the first four chunks are
                # split into y-halves issued in parallel on the scalar and
                # gpsimd queues while sync carries aux; the rest round-robin
                # in consumption order so no later chunk starves an earlier
                # one.
                w_tiles = [wpool.tile([128, YC], bf16, tag=f"w{k}",
                                      name=f"w{k}")
                           for k in range(KC)]
                for k in range(4):
                    nc.scalar.dma_start(
                        out=w_tiles[k][:, 0:YC // 2],
                        in_=w_d.ap()[k * 128:(k + 1) * 128, 0:YC // 2])
                    nc.gpsimd.dma_start(
                        out=w_tiles[k][:, YC // 2:YC],
                        in_=w_d.ap()[k * 128:(k + 1) * 128, YC // 2:YC])
                qeng = [nc.sync, nc.scalar, nc.gpsimd]
                for k in range(4, KC):
                    qeng[k % 3].dma_start(
                        out=w_tiles[k],
                        in_=w_d.ap()[k * 128:(k + 1) * 128, :])

                mask_tiles = [mpool.tile([128, FT], bf16, tag=f"m{k}",
                                         name=f"mask{k}")
                              for k in range(KC)]
                rm_sh = popool.tile([128, NYT * NCB], bf16, tag="rmsh")
                obuf_sb = popool.tile([128, 2 * NYT * NCB], bf16, tag="obuf")
                mg_view = obuf_sb[:, NYT * NCB:2 * NYT * NCB]

                def emit_mask(k, f, eng):
                    nsl = slice(f * NPF, (f + 1) * NPF)
                    t_b = tb_sb.unsqueeze(1).broadcast_to((128, NPF, T))
                    s_b = inT_sb[:, k * NCB:(k + 1) * NCB][:, nsl] \
                        .unsqueeze(2).broadcast_to((128, NPF, T))
                    eng.tensor_tensor(
                        mask_tiles[k][:, f * 512:(f + 1) * 512].rearrange(
                            "p (n t) -> p n t", n=NPF),
                        t_b, s_b, mybir.AluOpType.is_ge)

                def emit_mm(pt, k, yt, f, c0=0, ncols=512):
                    rhs = mask_tiles[k][:, f * 512 + c0:f * 512 + c0 + ncols]
                    lhsT = w_tiles[k][:, yt * 128:(yt + 1) * 128]
                    nc.tensor.matmul(pt, lhsT, rhs,
                                     start=(k == 0), stop=(k == KC - 1))

                def emit_post(pt, yt, f, c0=0, ncols=512):
                    nsub, noff = ncols // T, c0 // T
                    csl = slice(yt * NCB + f * NPF + noff,
                                yt * NCB + f * NPF + noff + nsub)
                    # rm = max_t (V > 1) * (T - t); 0 when never crossed
                    z_t = szpool.tile([128, ncols], bf16, tag="z")
                    r_b = revt_sb.unsqueeze(1).broadcast_to((128, nsub, T))
                    nc.vector.scalar_tensor_tensor(
                        z_t[:].rearrange("p (n t) -> p n t", n=nsub),
                        pt[:].rearrange("p (n t) -> p n t", n=nsub),
                        1.0, r_b,
                        mybir.AluOpType.is_gt, mybir.AluOpType.mult)
                    nc.vector.tensor_reduce(
                        rm_sh[:, csl],
                        z_t[:].rearrange("p (n t) -> p n t", n=nsub),
                        axis=mybir.AxisListType.X, op=mybir.AluOpType.max)
                    # margin = min_t |V - 1| on ACT + DVE
                    a_t = szpool.tile([128, ncols], bf16, tag="a")
                    nc.scalar.activation(a_t, pt,
                                         mybir.ActivationFunctionType.Abs,
                                         bias=neg1_sb[:])
                    nc.vector.tensor_reduce(
                        mg_view[:, csl],
                        a_t[:].rearrange("p (n t) -> p n t", n=nsub),
                        axis=mybir.AxisListType.X, op=mybir.AluOpType.min)

                # f0-half masks first (chunk order = consumption order),
                # f1 halves after; DVE stays ahead of the PE throughout.
                # (gpsimd rejects TENSOR_TENSOR is_ge, so all on DVE.)
                for k in range(KC):
                    emit_mask(k, 0, nc.vector)
                for k in range(KC):
                    emit_mask(k, 1, nc.vector)

                # f0 pass: k-outer so the PE trails the mask builder,
                # y-tiles 0..6 accumulate in 7 PSUM banks.
                pts = []
                for k in range(KC):
                    for yt in range(NYT - 1):
                        if k == 0:
                            pts.append(ps.tile([128, 512], f32, tag="pt",
                                               name=f"pt0_{yt}"))
                        emit_mm(pts[yt], k, yt, 0)
                # yt7-f0 rolls straight on (bank = recycled warmup slot)
                # while yts 0..6 postproc drains their banks; yt7-f1 then
                # lands in yt0's freed bank with no PE stall.
                pt7 = [ps.tile([128, 512], f32, tag="pt", name="pt7_0")]
                for k in range(KC):
                    emit_mm(pt7[0], k, NYT - 1, 0)
                for yt in range(NYT - 1):
                    emit_post(pts[yt], yt, 0)
                pt7.append(ps.tile([128, 512], f32, tag="pt", name="pt7_1"))
                for k in range(KC):
                    emit_mm(pt7[1], k, NYT - 1, 1)
                for f in range(2):
                    emit_post(pt7[f], NYT - 1, f)

                # f1 pass: y-outer so banks finish staggered and postproc
                # overlaps later y-tiles. The final y-tile runs as two
                # half-column banks so its first half's postproc overlaps
                # its second half's matmuls, shortening the exposed tail.
                for yt in range(NYT - 2):
                    pt = ps.tile([128, 512], f32, tag="pt", name=f"pt1_{yt}")
                    for k in range(KC):
                        emit_mm(pt, k, yt, 1)
                    emit_post(pt, yt, 1)
                ylast = NYT - 2
                for h in range(2):
                    pt = ps.tile([128, 256], f32, tag="pt", name=f"ptL_{h}")
                    for k in range(KC):
                        emit_mm(pt, k, ylast, 1, c0=h * 256, ncols=256)
                    emit_post(pt, ylast, 1, c0=h * 256, ncols=256)

                # out = 65 - max(rm, 1): crossed at t -> t+1, never -> 64.
                # DMA triggers chain on the producing engines (no cross-
                # engine semaphore hop); out and margins drain separately
                # so out doesn't wait on the final margin reduction.
                nc.vector.tensor_scalar_max(rm_sh[:], rm_sh[:], 1.0)
                nc.scalar.activation(
                    obuf_sb[:, 0:NYT * NCB], rm_sh[:],
                    mybir.ActivationFunctionType.Copy,
                    bias=float(T + 1), scale=-1.0)
                nc.scalar.dma_start(out=obuf_d.ap()[:, 0:NYT * NCB],
                                    in_=obuf_sb[:, 0:NYT * NCB])
                nc.sync.dma_start(out=obuf_d.ap()[:, NYT * NCB:],
                                  in_=mg_view)

    nc.compile()
    return nc


def _make_in_maps(inputs):
    import ml_dtypes

    input = np.ascontiguousarray(np.asarray(inputs["input"], dtype=np.float32))
    weight = np.ascontiguousarray(np.asarray(inputs["weight"], dtype=np.float32))
    t_series = np.asarray(inputs["t_series"], dtype=np.float32).reshape(-1)

    s_ceil = np.ceil(input).astype(np.float32)   # exact in bf16 (ints <= 64)
    TB = np.tile(t_series, (128, 1)).astype(np.float32)
    REVT = np.tile((np.float32(T) - np.arange(T, dtype=np.float32)), (128, 1))

    in_maps = []
    for c in range(8):
        yb, nb = c % Y_SH, c // Y_SH
        wsl = np.ascontiguousarray(
            weight[:, yb * YC:(yb + 1) * YC]).astype(ml_dtypes.bfloat16)
        scl = s_ceil[nb * NCB:(nb + 1) * NCB, :]          # (NCB, X)
        inT = scl.reshape(NCB, KC, 128).transpose(2, 1, 0).reshape(128, KC * NCB)
        aux = np.ascontiguousarray(
            np.concatenate([inT, TB, REVT], axis=1)).astype(ml_dtypes.bfloat16)
        in_maps.append({"aux": aux, "w": wsl})
    return in_maps


def kernel(input, weight, t_series, T=64, **unused):
    global LAST_RESULTS
    from concourse import bass_utils

    _ensure_ntff_hook()
    _safe_upload_artifacts()
    if "nc" not in _cache:
        _cache["nc"] = _build_nc()
    nc = _cache["nc"]

    _cache["t_series"] = np.asarray(t_series, dtype=np.float32).reshape(-1)
    in_maps = _make_in_maps(
        {"input": input, "weight": weight, "t_series": t_series})

    res = bass_utils.run_bass_kernel_spmd(
        nc, in_maps, core_ids=list(range(8)), trace=TRACE)
    LAST_RESULTS = res

    # device layout: obuf[p, yt*NCB + n] = out for y = yt*128+p, batch n;
    # columns NYT*NCB.. hold the margins in the same layout
    O = np.empty((YY, NN), dtype=np.float32)
    M = np.empty((YY, NN), dtype=np.float32)
    for c, r in enumerate(res.results):
        yb, nb = c % Y_SH, c // Y_SH
        ob = np.asarray(r["obuf"]).astype(np.float32).reshape(128, 2, NYT, NCB)
        O[yb * YC:(yb + 1) * YC, nb * NCB:(nb + 1) * NCB] = \
            ob[:, 0].transpose(1, 0, 2).reshape(YC, NCB)
        M[yb * YC:(yb + 1) * YC, nb * NCB:(nb + 1) * NCB] = \
            ob[:, 1].transpose(1, 0, 2).reshape(YC, NCB)
    out = np.ascontiguousarray(O.T)

    _host_fixup(out, M.T, np.asarray(input, np.float32),
                np.asarray(weight, np.float32))
    return out


def _host_fixup(out, margin, input, weight):
    """Recompute exactly (fp64) every element whose bf16 |V-1| margin is
    within the bf16 matmul error bound; in-place on `out`."""
    flags = margin < FIX_EPS
    if not flags.any():
        return
    # first step index j with t_series[j] >= in; == T means never spikes
    s = np.searchsorted(_cache.get("t_series", np.arange(T, dtype=np.float32)),
                        input, side="left").astype(np.int64)
    s = np.clip(s, 0, T)
    w64 = weight.astype(np.float64)
    for n in np.unique(np.nonzero(flags)[0]):
        ys = np.nonzero(flags[n])[0]
        d = np.zeros((T + 1, len(ys)))
        np.add.at(d, s[n], w64[:, ys])           # scatter rows by spike step
        V = np.cumsum(d[:T], axis=0)
        c = V > 1.0
        any_c = c.any(axis=0)
        idx = np.argmax(c, axis=0)
        out[n, ys] = np.where(any_c, idx + 1, T).astype(np.float32)
